# revision 1
# baseline (speedup 1.0000x reference)
"""TRN2 Bass/Tile kernel: nn_ChannelWiseTensorSquareSelfInteraction.

Contract: kernel(**inputs) takes the FULL unsharded inputs
(x [100000,512], mlp_w1 [384,384], mlp_w2 [384,768], lin_ws [384,128],
lin_wv [256,128], all fp32) and returns the FULL output [100000,512] fp32.

Strategy (8 NeuronCores, data-parallel over the node axis):
  - Host: pad nodes 100000 -> 8*12800, shard; de-interleave x into
    feature-major chunks [s | vx | vy | vz] each [128, nodes] so the
    device kernel needs no input transposes. Weights replicated; the
    sqrt(2) factor of the sv path and the unused 2e-gate columns of
    mlp_w2 are folded/dropped host-side.
  - Device (per core, feature-major: channels on partitions, nodes on
    the free dim, tiles of 512 nodes):
      products ss/vv (GPSIMD+DVE) -> mm1+silu -> mm2+silu (PE+ACT,
      float32r matmuls at full PE rate; weights and the silu/gating
      producers emit f32r directly) -> gating muls (DVE/GPSIMD) ->
      equivariant linear (PE) with the residual added exactly in fp32
      during PSUM evacuation (DVE scalar_tensor_tensor) -> PE transpose
      back to node-major -> LayerNorm with batched Newton-rsqrt
      denominators (keeps ACT on a single activation-table set) ->
      store node-major (the 1o interleave restored via a strided AP).
"""

import numpy as np

import concourse.bacc as bacc
import concourse.mybir as mybir
from concourse.tile import TileContext
from concourse.masks import make_identity
from concourse.bass_utils import run_bass_kernel_spmd

F32 = mybir.dt.float32
F32R = mybir.dt.float32r
F16 = mybir.dt.float16
AF = mybir.ActivationFunctionType
OP = mybir.AluOpType
EPS = 1e-6

N_FULL = 100000
N_CORES = 8
NPC = 12544  # padded nodes per core (24 tiles of 512 + 1 of 256)

# engine/pool tuning knobs (overridable for cost-model experiments)
OPTS = dict(
    ph_bufs=1, pg_bufs=2, po_bufs=1, pnm_bufs=4, pin_bufs=4, pout_bufs=3,
    conv_engine="act",      # producer of s_r/ss_r/vv_r copies: act|gps|dve
    gsv_engine="dve",       # gsv mul: dve|gps
    vg_split=1,             # how many of the 3 vg muls go to gps
    split_out=True,         # store each tile with two half-tile DMAs
    fp16=False,             # fp16 matmul-input pipeline (vs f32r)
)


def build_nc(npc: int = NPC, T: int = 512):
    """Build the per-core Bass program. npc = nodes per core."""
    assert npc % 256 == 0 and T % 128 == 0
    tile_sizes = []
    rem = npc
    while rem > 0:
        step = min(T, rem)
        tile_sizes.append(step)
        rem -= step

    FR = F16 if OPTS["fp16"] else F32R
    nc = bacc.Bacc("TRN2", target_bir_lowering=False, debug=False, num_devices=N_CORES)
    xt = nc.declare_dram_parameter("xt", [4, 128, npc], F32, isOutput=False)
    w1 = nc.declare_dram_parameter("w1", [384, 384], FR, isOutput=False)
    w2 = nc.declare_dram_parameter("w2", [384, 640], FR, isOutput=False)
    ws = nc.declare_dram_parameter("ws", [384, 128], FR, isOutput=False)
    wv = nc.declare_dram_parameter("wv", [256, 128], FR, isOutput=False)
    if OPTS["fp16"]:
        xt16 = nc.declare_dram_parameter("xt16", [4, 128, npc], F16, isOutput=False)
        xt16_r = xt16.rearrange("c p n -> p c n")
    elif OPTS.get("sr_dma", False):
        xs_r = nc.declare_dram_parameter("xs_r", [128, npc], F32R, isOutput=False)
    y = nc.declare_dram_parameter("y", [npc, 512], F32, isOutput=True)

    xt_r = xt.rearrange("c p n -> p c n")

    with TileContext(nc) as tc:
        with (
            tc.tile_pool(name="singles", bufs=1) as singles,
            tc.tile_pool(name="pin", bufs=OPTS.get("pin_bufs",3)) as pin,
            tc.tile_pool(name="pmid", bufs=OPTS.get("pmid_bufs",2)) as pmid,
            tc.tile_pool(name="pout", bufs=OPTS.get("pout_bufs",2)) as pout,
            tc.tile_pool(name="psmall", bufs=OPTS.get("psmall_bufs",3)) as psmall,
            tc.tile_pool(name="ph", bufs=OPTS["ph_bufs"], space="PSUM") as ph,
            tc.tile_pool(name="pg", bufs=OPTS["pg_bufs"], space="PSUM") as pg,
            tc.tile_pool(name="po", bufs=OPTS["po_bufs"], space="PSUM") as po,
            tc.tile_pool(name="pnm", bufs=OPTS["pnm_bufs"], space="PSUM") as pnm,
        ):
            # --- resident weights (host pre-rounded to the PE input format) ---
            w1_r = singles.tile([128, 3, 384], FR)
            nc.sync.dma_start(out=w1_r, in_=w1.rearrange("(k p) m -> p k m", p=128))
            w2_r = singles.tile([128, 3, 640], FR)
            nc.sync.dma_start(out=w2_r, in_=w2.rearrange("(k p) m -> p k m", p=128))
            ws_r = singles.tile([128, 3, 128], FR)
            nc.sync.dma_start(out=ws_r, in_=ws.rearrange("(k p) m -> p k m", p=128))
            wv_r = singles.tile([128, 2, 128], FR)
            nc.sync.dma_start(out=wv_r, in_=wv.rearrange("(k p) m -> p k m", p=128))
            ident = singles.tile([128, 128], F32)
            make_identity(nc, ident)
            ones8 = singles.tile([128, 8], F32)
            nc.vector.memset(ones8, 1.0)

            def mm(out_p, lhsT, rhs, start, stop):
                nc.tensor.matmul(out_p, lhsT, rhs, start=start, stop=stop)

            def stage_ab(ns, T):
                def conv(dst, srcv):
                    if OPTS["conv_engine"] == "act":
                        nc.scalar.activation(out=dst, in_=srcv, func=AF.Copy)
                    elif OPTS["conv_engine"] == "gps":
                        nc.gpsimd.tensor_copy(out=dst, in_=srcv)
                    else:
                        nc.vector.tensor_copy(out=dst, in_=srcv)

                xin = pin.tile([128, 4, T], F32, tag="xin")
                if OPTS.get("split_in", False):
                    nc.sync.dma_start(out=xin[:, 0:2, :], in_=xt_r[:, 0:2, ns])
                    nc.sync.dma_start(out=xin[:, 2:4, :], in_=xt_r[:, 2:4, ns])
                else:
                    nc.sync.dma_start(out=xin, in_=xt_r[:, :, ns])
                s = xin[:, 0, :]
                v3 = [xin[:, 1, :], xin[:, 2, :], xin[:, 3, :]]

                # --- channel-wise products (scal chunks in the PE input format) ---
                if OPTS["fp16"]:
                    x16 = pin.tile([128, 4, T], F16, tag="x16")
                    nc.sync.dma_start(out=x16, in_=xt16_r[:, :, ns])
                    s16 = x16[:, 0, :]
                    v16 = [x16[:, 1, :], x16[:, 2, :], x16[:, 3, :]]
                    ss16 = pmid.tile([128, T], F16, tag="ss16")
                    nc.gpsimd.tensor_mul(ss16, s16, s16)
                    sqx = pmid.tile([128, T], F16, tag="sqx", bufs=2)
                    nc.gpsimd.tensor_mul(sqx, v16[0], v16[0])
                    sqy = pmid.tile([128, T], F16, tag="sqy", bufs=2)
                    nc.gpsimd.tensor_mul(sqy, v16[1], v16[1])
                    sqz = pmid.tile([128, T], F16, tag="sqz", bufs=2)
                    nc.gpsimd.tensor_mul(sqz, v16[2], v16[2])
                    vvp = pmid.tile([128, T], F16, tag="vvp", bufs=2)
                    nc.vector.tensor_add(vvp, sqx, sqy)
                    vv16 = pmid.tile([128, T], F16, tag="vv16")
                    nc.vector.tensor_add(vv16, vvp, sqz)
                    scal_f = [s16, ss16, vv16]
                    scal_r = scal_f
                    gate_v = v16
                else:
                    s_r = pmid.tile([128, T], F32R, tag="s_r")
                    if OPTS.get("sr_dma", False):
                        nc.sync.dma_start(out=s_r, in_=xs_r[:, ns])
                    else:
                        conv(s_r, s)
                    ss_f = pmid.tile([128, T], F32, tag="ss_f")
                    nc.gpsimd.tensor_mul(ss_f, s, s)
                    ss_r = pmid.tile([128, T], F32R, tag="ss_r")
                    conv(ss_r, ss_f)
                    sqx = pmid.tile([128, T], F32, tag="sqx", bufs=2)
                    nc.gpsimd.tensor_mul(sqx, v3[0], v3[0])
                    sqy = pmid.tile([128, T], F32, tag="sqy", bufs=2)
                    nc.gpsimd.tensor_mul(sqy, v3[1], v3[1])
                    sqz = pmid.tile([128, T], F32, tag="sqz", bufs=2)
                    nc.gpsimd.tensor_mul(sqz, v3[2], v3[2])
                    vvp = pmid.tile([128, T], F32, tag="vvp", bufs=2)
                    nc.vector.tensor_add(vvp, sqx, sqy)
                    vv_f = pmid.tile([128, T], F32, tag="vv_f")
                    nc.vector.tensor_add(vv_f, vvp, sqz)
                    vv_r = pmid.tile([128, T], F32R, tag="vv_r")
                    conv(vv_r, vv_f)
                    scal_f = [s, ss_f, vv_f]
                    scal_r = [s_r, ss_r, vv_r]
                    gate_v = v3

                # --- MLP layer 1: hidden = silu(scal @ w1) ---
                h_sb = pmid.tile([128, 3, T], FR, tag="h")
                for m in range(3):
                    psum_h = ph.tile([128, T], F32, tag="ph")
                    for k in range(3):
                        mm(psum_h, w1_r[:, k, 128 * m : 128 * (m + 1)], scal_r[k],
                           start=(k == 0), stop=(k == 2))
                    nc.scalar.activation(out=h_sb[:, m, :], in_=psum_h, func=AF.Silu)

                # --- MLP layer 2: gates = silu(hidden @ w2[:, :640]) ---
                g_sb = pmid.tile([128, 5, T], F16 if OPTS["fp16"] else F32, tag="g")
                for m in range(5):
                    psum_g = pg.tile([128, T], F32, tag="pg")
                    for k in range(3):
                        mm(psum_g, w2_r[:, k, 128 * m : 128 * (m + 1)], h_sb[:, k, :],
                           start=(k == 0), stop=(k == 2))
                    nc.scalar.activation(out=g_sb[:, m, :], in_=psum_g, func=AF.Silu)
                gv1 = g_sb[:, 3, :]
                gv2 = g_sb[:, 4, :]

                # --- gating (elementwise, f32r outputs feed the PE) ---
                sg = pmid.tile([128, 3, T], FR, tag="sg")
                nc.vector.tensor_mul(sg[:, 0, :], scal_f[0], g_sb[:, 0, :])
                nc.vector.tensor_mul(sg[:, 1, :], scal_f[1], g_sb[:, 1, :])
                sg2_eng = nc.vector if OPTS.get("sg2_dve", False) else nc.gpsimd
                sg2_eng.tensor_mul(sg[:, 2, :], scal_f[2], g_sb[:, 2, :])
                vg = pmid.tile([128, 3, T], FR, tag="vg")
                for i in range(3):
                    eng = nc.gpsimd if i < OPTS["vg_split"] else nc.vector
                    eng.tensor_mul(vg[:, i, :], gate_v[i], gv1)
                gsv = pmid.tile([128, T], F16 if OPTS["fp16"] else F32, tag="gsv")
                (nc.gpsimd if OPTS["gsv_engine"] == "gps" else nc.vector).tensor_mul(gsv, scal_f[0], gv2)
                svg = pmid.tile([128, 3, T], FR, tag="svg")
                for i in range(3):
                    nc.gpsimd.tensor_mul(svg[:, i, :], gsv, gate_v[i])

                # --- equivariant linear; residual added exactly during evacuation ---
                oc_sb = pout.tile([128, 4, T], F32, tag="oc")
                lin_pool, lin_tag = (pg, "pg") if OPTS.get("merge_po", False) else (po, "po")
                pos = lin_pool.tile([128, T], F32, tag=lin_tag)
                for k in range(3):
                    mm(pos, ws_r[:, k, :], sg[:, k, :], start=(k == 0), stop=(k == 2))
                nc.vector.scalar_tensor_tensor(
                    out=oc_sb[:, 0, :], in0=pos, scalar=0.0, in1=s,
                    op0=OP.add, op1=OP.add,
                )
                for i in range(3):
                    pov = lin_pool.tile([128, T], F32, tag=lin_tag)
                    mm(pov, wv_r[:, 0, :], vg[:, i, :], start=True, stop=False)
                    mm(pov, wv_r[:, 1, :], svg[:, i, :], start=False, stop=True)
                    nc.vector.scalar_tensor_tensor(
                        out=oc_sb[:, 1 + i, :], in0=pov, scalar=0.0, in1=v3[i],
                        op0=OP.add, op1=OP.add,
                    )

                return oc_sb

            def stage_c(ns, T, oc_sb):
                NB = T // 128
                # --- transpose to node-major + LayerNorm + store ---
                # sq[:, 0, :] holds 128*var_s per block; sq[:, 1, :] holds
                # sum(v^2) per block. One batched Newton-rsqrt chain then
                # yields 1/(sqrt(var)+eps) for both parts.
                y_sb = pout.tile([128, NB, 512], F32, tag="y")
                stats = psmall.tile([128, NB, 6], F32, tag="stats")
                mv = psmall.tile([128, NB, 2], F32, tag="mv")
                sq = psmall.tile([128, 2, NB], F32, tag="sq")
                merged = OPTS.get("pnm_merge", False)
                if merged:
                    pnm_big = pnm.tile([128, NB, 512], F32, tag="pnmB", bufs=1)
                    pnm_ts = [pnm_big[:, b, :] for b in range(NB)]
                else:
                    pnm_ts = []
                for b in range(NB):
                    if merged:
                        pnm_t = pnm_ts[b]
                    else:
                        pnm_t = pnm.tile([128, 512], F32, tag="pnm")
                        pnm_ts.append(pnm_t)
                    for c in range(4):
                        nc.tensor.matmul(
                            pnm_t[:, 128 * c : 128 * (c + 1)],
                            oc_sb[:, c, 128 * b : 128 * (b + 1)],
                            ident,
                            is_transpose=True,
                        )
                    # scalar-part stats: mean/var over the 128 features
                    nc.vector.bn_stats(out=stats[:, b, :], in_=pnm_t[:, 0:128])
                    nc.vector.bn_aggr(out=mv[:, b, :], in_=stats[:, b, :])
                    # vector-part sumsq over all 384 components
                    vscr = pmid.tile([128, 384], F32, tag="vscr")
                    nc.scalar.activation(
                        out=vscr, in_=pnm_t[:, 128:512], func=AF.Square,
                        accum_out=sq[:, 1, b : b + 1],
                    )
                # gather 128*var_s (both halves of sq then hold 128*mean-square)
                NE = nc.gpsimd if OPTS.get("newton_gps", False) else nc.vector
                NE.tensor_scalar(
                    out=sq[:, 0, :], in0=mv[:, :, 1], scalar1=128.0, scalar2=None,
                    op0=OP.mult,
                )
                # inv = 1/(sqrt(w/128)+eps) via Newton rsqrt (keeps ACT on one
                # table set; exact to ~1e-5): seed = magic - (bits>>1), two
                # iterations y *= 1.5 - 0.5*w*y^2, then d=w*y, inv=1/(d+eps).
                w = psmall.tile([128, 2 * NB], F32, tag="nw")
                NE.tensor_scalar(
                    out=w, in0=sq.rearrange("p a b -> p (a b)"),
                    scalar1=1.0 / 128.0, scalar2=None, op0=OP.mult,
                )
                wi = w.bitcast(mybir.dt.int32)
                yv = psmall.tile([128, 2 * NB], F32, tag="ny")
                yi = yv.bitcast(mybir.dt.int32)
                NE.tensor_scalar(out=yi, in0=wi, scalar1=1, scalar2=None,
                                 op0=OP.arith_shift_right)
                NE.tensor_scalar(out=yi, in0=yi, scalar1=0x5F3759E0,
                                 scalar2=None, op0=OP.subtract)
                NE.tensor_scalar(out=yi, in0=yi, scalar1=-1, scalar2=None,
                                 op0=OP.bitwise_xor)
                hv = psmall.tile([128, 2 * NB], F32, tag="nh")
                NE.tensor_scalar(out=hv, in0=w, scalar1=0.5, scalar2=None,
                                 op0=OP.mult)
                tmp = psmall.tile([128, 2 * NB], F32, tag="nt")
                for _ in range(2):
                    NE.tensor_mul(tmp, yv, yv)
                    NE.tensor_mul(tmp, tmp, hv)
                    NE.tensor_scalar(out=tmp, in0=tmp, scalar1=-1.0,
                                     scalar2=1.5, op0=OP.mult, op1=OP.add)
                    NE.tensor_mul(yv, yv, tmp)
                den = psmall.tile([128, 2 * NB], F32, tag="nd")
                NE.tensor_mul(den, w, yv)
                NE.tensor_scalar_add(den, den, EPS)
                inv = psmall.tile([128, 2 * NB], F32, tag="ninv")
                if OPTS.get("newton_gps", False):
                    nc.gpsimd.tensor_tensor(
                        out=inv, in0=ones8, in1=den, op=OP.divide
                    )
                else:
                    nc.vector.reciprocal(inv, den)
                # bias for the scalar part: -mu * inv_s
                nbias = psmall.tile([128, NB], F32, tag="nbias")
                NE.scalar_tensor_tensor(
                    out=nbias, in0=mv[:, :, 0], scalar=-1.0, in1=inv[:, 0:NB],
                    op0=OP.mult, op1=OP.mult,
                )
                for b in range(NB):
                    pnm_t = pnm_ts[b]
                    # normalize on ACT: out = in*scale + bias (per-node scalars)
                    nc.scalar.activation(
                        out=y_sb[:, b, 0:128], in_=pnm_t[:, 0:128], func=AF.Identity,
                        bias=nbias[:, b : b + 1], scale=inv[:, b : b + 1],
                    )
                    if merged:
                        continue
                    vdst = y_sb[:, b, 128:512].rearrange("p (o i) -> p i o", i=3)
                    vsrc = pnm_t[:, 128:512].rearrange("p (i o) -> p i o", o=128)
                    if b < OPTS.get("tsv_act", 0):
                        nc.scalar.activation(
                            out=vdst, in_=vsrc, func=AF.Copy,
                            scale=inv[:, NB + b : NB + b + 1],
                        )
                    else:
                        nc.vector.tensor_scalar(
                            out=vdst, in0=vsrc,
                            scalar1=inv[:, NB + b : NB + b + 1], scalar2=None,
                            op0=OP.mult,
                        )
                if merged:
                    # single broadcast multiply over all blocks' vector parts
                    vdst = y_sb[:, :, 128:512].rearrange("p b (o i) -> p b i o", i=3)
                    vsrc = pnm_big[:, :, 128:512].rearrange("p b (i o) -> p b i o", o=128)
                    invb = inv[:, NB : 2 * NB].rearrange("p b -> p b () ()").broadcast_to(
                        (128, NB, 3, 128)
                    )
                    nc.vector.tensor_mul(vdst, vsrc, invb)
                y_blk = y[ns].rearrange("(b p) f -> p b f", p=128)
                if OPTS.get("split_out4", False):
                    for b in range(NB):
                        nc.sync.dma_start(out=y_blk[:, b : b + 1, :], in_=y_sb[:, b : b + 1, :])
                elif OPTS.get("split_out", False):
                    h2 = NB // 2
                    nc.sync.dma_start(out=y_blk[:, 0:h2, :], in_=y_sb[:, 0:h2, :])
                    nc.sync.dma_start(out=y_blk[:, h2:NB, :], in_=y_sb[:, h2:NB, :])
                else:
                    nc.sync.dma_start(out=y_blk, in_=y_sb)

            off = 0
            for Tt in tile_sizes:
                ns = slice(off, off + Tt)
                oc = stage_ab(ns, Tt)
                stage_c(ns, Tt, oc)
                off += Tt

    nc.finalize()
    return nc


def _round_f32r(a):
    """Round fp32 to the PE's f32r precision (11 explicit mantissa bits)."""
    i = np.ascontiguousarray(a, np.float32).view(np.int32)
    r = ((i + 0x7FF + ((i >> 12) & 1)) >> 12) << 12
    return r.astype(np.int32).view(np.float32)


def host_prep(x_full, mlp_w1, mlp_w2, lin_ws, lin_wv, npc: int = NPC):
    """Pad + shard + feature-major de-interleave. Returns 8 input maps."""
    x_full = np.asarray(x_full, np.float32)
    n = x_full.shape[0]
    xp = np.zeros((N_CORES * npc, 512), dtype=np.float32)
    xp[:n] = x_full
    w1 = _round_f32r(np.asarray(mlp_w1, np.float32))
    w2 = _round_f32r(np.asarray(mlp_w2, np.float32)[:, :640])
    ws_ = _round_f32r(np.asarray(lin_ws, np.float32))
    wv_np = np.asarray(lin_wv, np.float32)
    wv_ = _round_f32r(
        np.concatenate([wv_np[:128], np.float32(np.sqrt(2.0)) * wv_np[128:]], axis=0)
    )
    fp16 = OPTS["fp16"]
    if fp16:
        w1, w2, ws_, wv_ = (a.astype(np.float16) for a in (w1, w2, ws_, wv_))
    maps = []
    for c in range(N_CORES):
        xs = xp[c * npc : (c + 1) * npc]
        xtc = np.empty((4, 128, npc), dtype=np.float32)
        xtc[0] = xs[:, :128].T
        v = xs[:, 128:].reshape(npc, 128, 3)
        xtc[1] = v[:, :, 0].T
        xtc[2] = v[:, :, 1].T
        xtc[3] = v[:, :, 2].T
        m = dict(xt=xtc, w1=w1, w2=w2, ws=ws_, wv=wv_)
        if fp16:
            m["xt16"] = xtc.astype(np.float16)
        elif OPTS.get("sr_dma", False):
            m["xs_r"] = _round_f32r(xtc[0])
        maps.append(m)
    return maps


_CACHE = {}


def _get_nc():
    if "nc" not in _CACHE:
        _CACHE["nc"] = build_nc()
    return _CACHE["nc"]


def kernel(x, mlp_w1, mlp_w2, lin_ws, lin_wv):
    maps = host_prep(x, mlp_w1, mlp_w2, lin_ws, lin_wv)
    nc = _get_nc()
    res = run_bass_kernel_spmd(nc, maps, list(range(N_CORES)))
    n = np.asarray(x).shape[0]
    out = np.concatenate([res.results[c]["y"] for c in range(N_CORES)], axis=0)[:n]
    return np.ascontiguousarray(out)


def timed_stats():
    """Extra (test-only) instrumentation: simulated per-core exec time."""
    try:
        from concourse.timeline_sim import TimelineSim

        sim = TimelineSim(_get_nc())
        return float(sim.simulate())
    except Exception as e:  # pragma: no cover
        print("timeline sim failed:", e)
        return None



# revision 2
# speedup vs baseline: 1.0031x; 1.0031x over previous
"""TRN2 Bass/Tile kernel v2: nn_ChannelWiseTensorSquareSelfInteraction.

Contract: kernel(**inputs) takes FULL unsharded fp32 inputs
(x [100000,512], mlp_w1 [384,384], mlp_w2 [384,768], lin_ws [384,128],
lin_wv [256,128]) and returns the FULL fp32 output [100000,512].

Strategy (8 cores, data-parallel over nodes, fp16 device pipeline):
  Host ships fp16 feature-major x (s|vx|vy|vz), fp16 products (ss, vv),
  fp16 weights pre-chunked for the PE, fp16 identity; reads back fp16 y.
  Device per 512-node tile:
    mm1 (fp16) -> silu_h (ACT, one call) -> mm2 (fp16, m-split 3+2)
    -> silu_g -> gating muls (DVE + Pool) -> node-major linear with
    residual via identity-rhs matmuls (PE) -> evac (ACT s-part copy,
    DVE v-part interleaving copy) -> stats (DVE: grouped bn_stats for s,
    tensor_tensor_reduce for v sumsq) -> Newton rsqrt + stat algebra
    (Pool, batched) -> per-block (raw-mu)*inv via tensor_scalar with
    per-partition scalar APs (DVE fast mode) -> fp16 store.
  Emission is software-pipelined (mm1 of tile t+1 before mm2 of tile t
  before lin of tile t-1) so the PE — the bottleneck engine at fp16 —
  never waits on ACT/DVE stages.
"""

import numpy as np

import concourse.bacc as bacc
import concourse.mybir as mybir
from concourse.tile import TileContext
from concourse.bass_utils import run_bass_kernel_spmd

F32 = mybir.dt.float32
F16 = mybir.dt.float16
I32 = mybir.dt.int32
AF = mybir.ActivationFunctionType
OP = mybir.AluOpType

N_FULL = 100000
N_CORES = 8
NPC = 12800       # 25 tiles of 512
T = 512
NB = 4            # node blocks per tile
NT = NPC // T

MAGIC = 0x5F3759DF

OPTS = dict(
    newton_iters=1,
    svg_pool=3,        # how many of the 3 svg muls run on Pool
    evac_s_act=True,   # s-part evacuation on ACT (else DVE)
)


def build_nc():
    nc = bacc.Bacc("TRN2", target_bir_lowering=False, debug=False, num_devices=N_CORES)
    xt = nc.declare_dram_parameter("xt", [6, 128, NPC], F16, isOutput=False)
    w1 = nc.declare_dram_parameter("w1", [128, 3, 384], F16, isOutput=False)
    w2 = nc.declare_dram_parameter("w2", [128, 3, 640], F16, isOutput=False)
    ws = nc.declare_dram_parameter("ws", [128, 3, 128], F16, isOutput=False)
    wv = nc.declare_dram_parameter("wv", [128, 2, 128], F16, isOutput=False)
    idn = nc.declare_dram_parameter("idn", [128, 128], F16, isOutput=False)
    y = nc.declare_dram_parameter("y", [NPC, 512], F16, isOutput=True)

    xt_r = xt.rearrange("c p n -> p c n")

    with TileContext(nc) as tc:
        with (
            tc.tile_pool(name="singles", bufs=1) as singles,
            tc.tile_pool(name="pin", bufs=5) as pin,
            tc.tile_pool(name="pmid", bufs=3) as pmid,
            tc.tile_pool(name="pout", bufs=4) as pout,
            tc.tile_pool(name="psmall", bufs=4) as psmall,
            tc.tile_pool(name="phg", bufs=2, space="PSUM") as phg,
            tc.tile_pool(name="po", bufs=2, space="PSUM") as po,
        ):
            w1_r = singles.tile([128, 3, 384], F16)
            w2_r = singles.tile([128, 3, 640], F16)
            ws_r = singles.tile([128, 3, 128], F16)
            wv_r = singles.tile([128, 2, 128], F16)
            ident = singles.tile([128, 128], F16)
            warm = singles.tile([128, 1], F32)
            nc.vector.memset(warm, 0.0)
            nc.scalar.activation(out=warm, in_=warm, func=AF.Silu)

            # pipeline state, keyed by tile index
            st = {}

            def S_in(t):
                ns = slice(t * T, (t + 1) * T)
                xin = pin.tile([128, 6, T], F16, tag="xin")
                nc.sync.dma_start(out=xin, in_=xt_r[:, :, ns])
                st[t] = dict(xin=xin)

            def S_mm1(t):
                d = st[t]
                xin = d["xin"]
                rhs = [xin[:, 0, :], xin[:, 1, :], xin[:, 2, :]]
                ph = phg.tile([128, 3, T], F32, tag="hg")
                for m in range(3):
                    for k in range(3):
                        nc.tensor.matmul(
                            ph[:, m, :], w1_r[:, k, 128 * m : 128 * (m + 1)], rhs[k],
                            start=(k == 0), stop=(k == 2),
                        )
                h16 = pmid.tile([128, 3, T], F16, tag="h16")
                nc.scalar.activation(
                    out=h16.rearrange("p a b -> p (a b)"),
                    in_=ph.rearrange("p a b -> p (a b)"),
                    func=AF.Silu,
                )
                d["h16"] = h16

            def S_mm2a(t):
                d = st[t]
                h16 = d["h16"]
                g16 = pmid.tile([128, 5, T], F16, tag="g16")
                d["g16"] = g16
                pga = phg.tile([128, 3, T], F32, tag="hg")
                for m in range(3):
                    for k in range(3):
                        nc.tensor.matmul(
                            pga[:, m, :], w2_r[:, k, 128 * m : 128 * (m + 1)],
                            h16[:, k, :], start=(k == 0), stop=(k == 2),
                        )
                nc.scalar.activation(
                    out=g16[:, 0:3, :].rearrange("p a b -> p (a b)"),
                    in_=pga.rearrange("p a b -> p (a b)"),
                    func=AF.Silu,
                )

            def S_mm2b(t):
                d = st[t]
                h16 = d["h16"]
                g16 = d["g16"]
                pgb = phg.tile([128, 3, T], F32, tag="hg")
                for m in range(2):
                    for k in range(3):
                        nc.tensor.matmul(
                            pgb[:, m, :], w2_r[:, k, 128 * (3 + m) : 128 * (4 + m)],
                            h16[:, k, :], start=(k == 0), stop=(k == 2),
                        )
                nc.scalar.activation(
                    out=g16[:, 3:5, :].rearrange("p a b -> p (a b)"),
                    in_=pgb[:, 0:2, :].rearrange("p a b -> p (a b)"),
                    func=AF.Silu,
                )

            def S_gate(t):
                d = st[t]
                xin, g16 = d["xin"], d["g16"]
                s = xin[:, 0, :]
                v3 = xin[:, 3:6, :]
                sg = pmid.tile([128, 3, T], F16, tag="sg")
                nc.vector.tensor_mul(sg, xin[:, 0:3, :], g16[:, 0:3, :])
                gsv = pmid.tile([128, T], F16, tag="gsv")
                nc.vector.tensor_mul(gsv, s, g16[:, 4, :])
                vg = pmid.tile([128, 3, T], F16, tag="vg")
                for i in range(3):
                    nc.vector.tensor_mul(vg[:, i, :], xin[:, 3 + i, :], g16[:, 3, :])
                svg = pmid.tile([128, 3, T], F16, tag="svg")
                for i in range(3):
                    nc.gpsimd.tensor_mul(svg[:, i, :], gsv, xin[:, 3 + i, :])
                d["sg"], d["vg"], d["svg"] = sg, vg, svg

            def S_lin_half(t, half):
                d = st[t]
                xin, sg, vg, svg = d["xin"], d["sg"], d["vg"], d["svg"]
                if half == 0:
                    y_sb = pout.tile([128, NB, 512], F16, tag="y")
                    d["y_sb"] = y_sb
                else:
                    y_sb = d["y_sb"]
                for b in (2 * half, 2 * half + 1):
                    cols = slice(128 * b, 128 * (b + 1))
                    pos = po.tile([128, 512], F32, tag="po")
                    # scalar part + residual (identity rhs)
                    for k in range(3):
                        nc.tensor.matmul(
                            pos[:, 0:128], sg[:, k, cols], ws_r[:, k, :],
                            start=(k == 0), stop=False,
                        )
                    nc.tensor.matmul(
                        pos[:, 0:128], xin[:, 0, cols], ident,
                        start=False, stop=True,
                    )
                    # vector part (comp-major in psum) + residual
                    for i in range(3):
                        dst = pos[:, 128 * (1 + i) : 128 * (2 + i)]
                        nc.tensor.matmul(dst, vg[:, i, cols], wv_r[:, 0, :],
                                         start=True, stop=False)
                        nc.tensor.matmul(dst, svg[:, i, cols], wv_r[:, 1, :],
                                         start=False, stop=False)
                        nc.tensor.matmul(dst, xin[:, 3 + i, cols], ident,
                                         start=False, stop=True)
                    # evacuate raw values to SBUF fp16; alternate the v-part
                    # engine (DVE/ACT) to balance load
                    vdst = y_sb[:, b, 128:512].rearrange("p (o i) -> p i o", i=3)
                    vsrc = pos[:, 128:512].rearrange("p (i o) -> p i o", o=128)
                    nc.scalar.activation(
                        out=y_sb[:, b, 0:128], in_=pos[:, 0:128], func=AF.Copy,
                    )
                    nc.scalar.activation(out=vdst, in_=vsrc, func=AF.Copy)

            def S_stat(t, half):
                d = st[t]
                y_sb = d["y_sb"]
                if half == 0:
                    # w[:, 0:NB] = var_s (filled in S_pmath); w[:, NB:] = msv
                    nw = psmall.tile([128, 2 * NB], F32, tag="nw")
                    st6 = psmall.tile([128, NB, 6], F32, tag="st6")
                    st6v = psmall.tile([128, NB, 6], F32, tag="st6v")
                    d["w"], d["stats6"], d["stats6v"] = nw, st6, st6v
                w, stats6, stat6v = d["w"], d["stats6"], d["stats6v"]
                for b in (2 * half, 2 * half + 1):
                    nc.vector.bn_stats(out=stat6v[:, b, :], in_=y_sb[:, b, 128:512])
                    nc.vector.bn_stats(out=stats6[:, b, :], in_=y_sb[:, b, 0:128])

            def S_pmath(t):
                d = st[t]
                w, stats6, stat6v = d["w"], d["stats6"], d["stats6v"]
                v = nc.vector
                # msv = sumsq_v/128 = (cve+cvo)/128 + 1.5*(me^2+mo^2)
                vme = stat6v[:, :, 1]
                vmo = stat6v[:, :, 4]
                vcve = stat6v[:, :, 2]
                vcvo = stat6v[:, :, 5]
                t1 = psmall.tile([128, NB], F32, tag="t1")
                v.tensor_mul(t1, vme, vme)
                t2 = psmall.tile([128, NB], F32, tag="t2")
                v.tensor_mul(t2, vmo, vmo)
                v.tensor_add(t1, t1, t2)
                v.tensor_scalar(out=t1, in0=t1, scalar1=1.5, scalar2=None,
                                op0=OP.mult)
                t3 = psmall.tile([128, NB], F32, tag="t3")
                v.tensor_add(t3, vcve, vcvo)
                v.scalar_tensor_tensor(out=w[:, NB : 2 * NB], in0=t3,
                                       scalar=1.0 / 128.0, in1=t1,
                                       op0=OP.mult, op1=OP.add)
                me = stats6[:, :, 1]
                mo = stats6[:, :, 4]
                cve = stats6[:, :, 2]
                cvo = stats6[:, :, 5]
                mu2 = psmall.tile([128, NB], F32, tag="mu2")
                v.tensor_add(mu2, me, mo)                 # 2*mu
                dh = psmall.tile([128, NB], F32, tag="dh")
                v.tensor_sub(dh, me, mo)
                v.tensor_scalar(out=dh, in0=dh, scalar1=0.5, scalar2=None,
                                op0=OP.mult)
                d2 = psmall.tile([128, NB], F32, tag="d2")
                v.tensor_mul(d2, dh, dh)
                cv = psmall.tile([128, NB], F32, tag="cv")
                v.tensor_add(cv, cve, cvo)
                # w[:, 0:NB] = var_s = cv/128 + d2 (w[:, NB:] set by ttr)
                v.scalar_tensor_tensor(out=w[:, 0:NB], in0=cv, scalar=1.0 / 128.0,
                                       in1=d2, op0=OP.mult, op1=OP.add)
                # Newton rsqrt seed (int bit trick)
                yv = psmall.tile([128, 2 * NB], F32, tag="ny")
                yi = yv.bitcast(I32)
                wi = w.bitcast(I32)
                v.tensor_scalar(out=yi, in0=wi, scalar1=1, scalar2=None,
                                op0=OP.arith_shift_right)
                v.tensor_scalar(out=yi, in0=yi, scalar1=0x5F3759E0,
                                scalar2=None, op0=OP.subtract)
                v.tensor_scalar(out=yi, in0=yi, scalar1=-1, scalar2=None,
                                op0=OP.bitwise_xor)
                tmp = psmall.tile([128, 2 * NB], F32, tag="nt")
                for _ in range(OPTS["newton_iters"]):
                    v.tensor_mul(tmp, yv, yv)
                    v.tensor_mul(tmp, tmp, w)
                    v.tensor_scalar(out=tmp, in0=tmp, scalar1=-0.5, scalar2=1.5,
                                    op0=OP.mult, op1=OP.add)
                    v.tensor_mul(yv, yv, tmp)
                # beta = -mu*inv_s = -0.5*mu2*inv_s
                beta = psmall.tile([128, NB], F32, tag="beta")
                v.scalar_tensor_tensor(out=beta, in0=mu2, scalar=-0.5,
                                       in1=yv[:, 0:NB], op0=OP.mult, op1=OP.mult)
                d["beta"], d["inv"] = beta, yv

            def S_fin(t):
                d = st[t]
                y_sb, beta, inv = d["y_sb"], d["beta"], d["inv"]
                for b in range(NB):
                    nc.vector.tensor_scalar(
                        out=y_sb[:, b, 0:128], in0=y_sb[:, b, 0:128],
                        scalar1=inv[:, b : b + 1], scalar2=None, op0=OP.mult,
                    )
                    nc.vector.tensor_scalar(
                        out=y_sb[:, b, 0:128], in0=y_sb[:, b, 0:128],
                        scalar1=beta[:, b : b + 1], scalar2=None, op0=OP.add,
                    )
                    nc.vector.tensor_scalar(
                        out=y_sb[:, b, 128:512], in0=y_sb[:, b, 128:512],
                        scalar1=inv[:, NB + b : NB + b + 1], scalar2=None,
                        op0=OP.mult,
                    )

            def S_out(t):
                d = st.pop(t)
                ns = slice(t * T, (t + 1) * T)
                y_blk = y[ns].rearrange("(b p) f -> p b f", p=128)
                nc.sync.dma_start(out=y_blk, in_=d["y_sb"])

            nc.sync.dma_start(out=w1_r, in_=w1[:, :, :])
            S_in(0)
            nc.sync.dma_start(out=w2_r, in_=w2[:, :, :])
            S_in(1)
            nc.sync.dma_start(out=ws_r, in_=ws[:, :, :])
            nc.sync.dma_start(out=wv_r, in_=wv[:, :, :])
            nc.sync.dma_start(out=ident, in_=idn[:, :])
            S_mm1(0)
            for i in range(NT + 2):
                if i + 2 <= NT - 1:
                    S_in(i + 2)
                if i + 1 <= NT - 1:
                    S_mm1(i + 1)
                if i <= NT - 1:
                    S_mm2a(i)
                if 0 <= i - 1 <= NT - 1:
                    S_lin_half(i - 1, 0)
                if i <= NT - 1:
                    S_mm2b(i)
                if 0 <= i - 1 <= NT - 1:
                    S_lin_half(i - 1, 1)
                    S_stat(i - 1, 0)
                if i <= NT - 1:
                    S_gate(i)
                if 0 <= i - 1 <= NT - 1:
                    S_stat(i - 1, 1)
                    S_pmath(i - 1)
                if 0 <= i - 2 <= NT - 1:
                    S_fin(i - 2)
                    S_out(i - 2)

    nc.finalize()
    return nc


def host_prep(x_full, mlp_w1, mlp_w2, lin_ws, lin_wv):
    x_full = np.asarray(x_full, np.float32)
    n = x_full.shape[0]
    xp = np.zeros((N_CORES * NPC, 512), dtype=np.float32)
    xp[:n] = x_full

    w1 = np.asarray(mlp_w1, np.float32)
    w2 = np.asarray(mlp_w2, np.float32)[:, :640]
    ws_ = np.asarray(lin_ws, np.float32)
    wv_np = np.asarray(lin_wv, np.float32)
    wv_ = np.concatenate(
        [wv_np[:128], np.float32(np.sqrt(2.0)) * wv_np[128:]], axis=0
    )
    # pre-chunk weights along K into [128, k, m] fp16
    w1_r = np.ascontiguousarray(w1.reshape(3, 128, 384).transpose(1, 0, 2)).astype(np.float16)
    w2_r = np.ascontiguousarray(w2.reshape(3, 128, 640).transpose(1, 0, 2)).astype(np.float16)
    ws_r = np.ascontiguousarray(ws_.reshape(3, 128, 128).transpose(1, 0, 2)).astype(np.float16)
    wv_r = np.ascontiguousarray(wv_.reshape(2, 128, 128).transpose(1, 0, 2)).astype(np.float16)
    idn = np.eye(128, dtype=np.float16)

    maps = []
    for c in range(N_CORES):
        xs = xp[c * NPC : (c + 1) * NPC]
        s = xs[:, :128]
        v = xs[:, 128:].reshape(NPC, 128, 3)
        s16 = s.T.astype(np.float16).astype(np.float32)
        v16 = v.astype(np.float16).astype(np.float32)
        xtc = np.empty((6, 128, NPC), dtype=np.float16)
        xtc[0] = s.T
        xtc[1] = (s16 * s16).astype(np.float16)
        xtc[2] = np.sum(v16 * v16, axis=-1).T.astype(np.float16)
        xtc[3] = v[:, :, 0].T
        xtc[4] = v[:, :, 1].T
        xtc[5] = v[:, :, 2].T
        maps.append(dict(xt=xtc, w1=w1_r, w2=w2_r, ws=ws_r, wv=wv_r, idn=idn))
    return maps


_CACHE = {}


def _get_nc():
    if "nc" not in _CACHE:
        _CACHE["nc"] = build_nc()
    return _CACHE["nc"]


def kernel(x, mlp_w1, mlp_w2, lin_ws, lin_wv):
    maps = host_prep(x, mlp_w1, mlp_w2, lin_ws, lin_wv)
    nc = _get_nc()
    res = run_bass_kernel_spmd(nc, maps, list(range(N_CORES)))
    n = np.asarray(x).shape[0]
    out = np.concatenate(
        [res.results[c]["y"] for c in range(N_CORES)], axis=0
    )[:n].astype(np.float32)
    return np.ascontiguousarray(out)


def timed_stats():
    try:
        from concourse.timeline_sim import TimelineSim

        sim = TimelineSim(_get_nc())
        return float(sim.simulate())
    except Exception as e:  # pragma: no cover
        print("timeline sim failed:", e)
        return None


# revision 3
# speedup vs baseline: 1.0261x; 1.0228x over previous
"""TRN2 Bass/Tile kernel v2: nn_ChannelWiseTensorSquareSelfInteraction.

Contract: kernel(**inputs) takes FULL unsharded fp32 inputs
(x [100000,512], mlp_w1 [384,384], mlp_w2 [384,768], lin_ws [384,128],
lin_wv [256,128]) and returns the FULL fp32 output [100000,512].

Strategy (8 cores, data-parallel over nodes, fp16 device pipeline):
  Host ships fp16 feature-major x (s|vx|vy|vz), fp16 products (ss, vv),
  fp16 weights pre-chunked for the PE, fp16 identity; reads back fp16 y.
  Device per 512-node tile:
    mm1 (fp16) -> silu_h (ACT, one call) -> mm2 (fp16, m-split 3+2)
    -> silu_g -> gating muls (DVE + Pool) -> node-major linear with
    residual via identity-rhs matmuls (PE) -> evac (ACT s-part copy,
    DVE v-part interleaving copy) -> stats (DVE: grouped bn_stats for s,
    tensor_tensor_reduce for v sumsq) -> Newton rsqrt + stat algebra
    (Pool, batched) -> per-block (raw-mu)*inv via tensor_scalar with
    per-partition scalar APs (DVE fast mode) -> fp16 store.
  Emission is software-pipelined (mm1 of tile t+1 before mm2 of tile t
  before lin of tile t-1) so the PE — the bottleneck engine at fp16 —
  never waits on ACT/DVE stages.
"""

import numpy as np

import concourse.bacc as bacc
import concourse.mybir as mybir
from concourse.tile import TileContext
from concourse.bass_utils import run_bass_kernel_spmd

F32 = mybir.dt.float32
F16 = mybir.dt.float16
I32 = mybir.dt.int32
AF = mybir.ActivationFunctionType
OP = mybir.AluOpType

N_FULL = 100000
N_CORES = 8
NPC = 12800       # 25 tiles of 512
T = 512
NB = 4            # node blocks per tile
NT = NPC // T

MAGIC = 0x5F3759DF

OPTS = dict(
    newton_iters=1,
    svg_pool=3,        # how many of the 3 svg muls run on Pool
    evac_s_act=True,   # s-part evacuation on ACT (else DVE)
)


def build_nc():
    nc = bacc.Bacc("TRN2", target_bir_lowering=False, debug=False, num_devices=N_CORES)
    xt = nc.declare_dram_parameter("xt", [6, 128, NPC], F16, isOutput=False)
    w1 = nc.declare_dram_parameter("w1", [128, 3, 384], F16, isOutput=False)
    w2 = nc.declare_dram_parameter("w2", [128, 3, 640], F16, isOutput=False)
    ws = nc.declare_dram_parameter("ws", [128, 3, 128], F16, isOutput=False)
    wv = nc.declare_dram_parameter("wv", [128, 2, 128], F16, isOutput=False)
    idn = nc.declare_dram_parameter("idn", [128, 128], F16, isOutput=False)
    y = nc.declare_dram_parameter("y", [NPC, 512], F16, isOutput=True)

    xt_r = xt.rearrange("c p n -> p c n")

    with TileContext(nc) as tc:
        with (
            tc.tile_pool(name="singles", bufs=1) as singles,
            tc.tile_pool(name="pin", bufs=5) as pin,
            tc.tile_pool(name="pmid", bufs=3) as pmid,
            tc.tile_pool(name="pout", bufs=4) as pout,
            tc.tile_pool(name="psmall", bufs=4) as psmall,
            tc.tile_pool(name="phg", bufs=2, space="PSUM") as phg,
            tc.tile_pool(name="po", bufs=2, space="PSUM") as po,
        ):
            w1_r = singles.tile([128, 3, 384], F16)
            w2_r = singles.tile([128, 3, 640], F16)
            ws_r = singles.tile([128, 3, 128], F16)
            wv_r = singles.tile([128, 2, 128], F16)
            ident = singles.tile([128, 128], F16)
            warm = singles.tile([128, 1], F32)
            nc.vector.memset(warm, 0.0)
            nc.scalar.activation(out=warm, in_=warm, func=AF.Silu)

            # pipeline state, keyed by tile index
            st = {}

            def S_in(t, split=False):
                ns = slice(t * T, (t + 1) * T)
                xin = pin.tile([128, 6, T], F16, tag="xin")
                if split:
                    nc.sync.dma_start(out=xin[:, 0:3, :], in_=xt_r[:, 0:3, ns])
                    st[t] = dict(xin=xin, pending=ns)
                else:
                    nc.sync.dma_start(out=xin, in_=xt_r[:, :, ns])
                    st[t] = dict(xin=xin)

            def S_mm1(t):
                d = st[t]
                xin = d["xin"]
                rhs = [xin[:, 0, :], xin[:, 1, :], xin[:, 2, :]]
                ph = phg.tile([128, 3, T], F32, tag="hg")
                for m in range(3):
                    for k in range(3):
                        nc.tensor.matmul(
                            ph[:, m, :], w1_r[:, k, 128 * m : 128 * (m + 1)], rhs[k],
                            start=(k == 0), stop=(k == 2),
                        )
                h16 = pmid.tile([128, 3, T], F16, tag="h16")
                nc.scalar.activation(
                    out=h16.rearrange("p a b -> p (a b)"),
                    in_=ph.rearrange("p a b -> p (a b)"),
                    func=AF.Silu,
                )
                d["h16"] = h16

            def S_mm2a(t):
                d = st[t]
                h16 = d["h16"]
                g16 = pmid.tile([128, 5, T], F16, tag="g16")
                d["g16"] = g16
                pga = phg.tile([128, 3, T], F32, tag="hg")
                for m in range(3):
                    for k in range(3):
                        nc.tensor.matmul(
                            pga[:, m, :], w2_r[:, k, 128 * m : 128 * (m + 1)],
                            h16[:, k, :], start=(k == 0), stop=(k == 2),
                        )
                nc.scalar.activation(
                    out=g16[:, 0:3, :].rearrange("p a b -> p (a b)"),
                    in_=pga.rearrange("p a b -> p (a b)"),
                    func=AF.Silu,
                )

            def S_mm2b(t):
                d = st[t]
                h16 = d["h16"]
                g16 = d["g16"]
                pgb = phg.tile([128, 3, T], F32, tag="hg")
                for m in range(2):
                    for k in range(3):
                        nc.tensor.matmul(
                            pgb[:, m, :], w2_r[:, k, 128 * (3 + m) : 128 * (4 + m)],
                            h16[:, k, :], start=(k == 0), stop=(k == 2),
                        )
                nc.scalar.activation(
                    out=g16[:, 3:5, :].rearrange("p a b -> p (a b)"),
                    in_=pgb[:, 0:2, :].rearrange("p a b -> p (a b)"),
                    func=AF.Silu,
                )

            def S_gate(t):
                d = st[t]
                xin, g16 = d["xin"], d["g16"]
                s = xin[:, 0, :]
                v3 = xin[:, 3:6, :]
                sg = pmid.tile([128, 3, T], F16, tag="sg")
                nc.vector.tensor_mul(sg, xin[:, 0:3, :], g16[:, 0:3, :])
                gsv = pmid.tile([128, T], F16, tag="gsv")
                nc.vector.tensor_mul(gsv, s, g16[:, 4, :])
                vg = pmid.tile([128, 3, T], F16, tag="vg")
                for i in range(3):
                    nc.vector.tensor_mul(vg[:, i, :], xin[:, 3 + i, :], g16[:, 3, :])
                svg = pmid.tile([128, 3, T], F16, tag="svg")
                for i in range(3):
                    nc.gpsimd.tensor_mul(svg[:, i, :], gsv, xin[:, 3 + i, :])
                d["sg"], d["vg"], d["svg"] = sg, vg, svg

            def S_lin_half(t, half):
                d = st[t]
                xin, sg, vg, svg = d["xin"], d["sg"], d["vg"], d["svg"]
                if half == 0:
                    y_sb = pout.tile([128, NB, 512], F16, tag="y")
                    d["y_sb"] = y_sb
                else:
                    y_sb = d["y_sb"]
                for b in (2 * half, 2 * half + 1):
                    cols = slice(128 * b, 128 * (b + 1))
                    pos = po.tile([128, 512], F32, tag="po")
                    # scalar part + residual (identity rhs)
                    for k in range(3):
                        nc.tensor.matmul(
                            pos[:, 0:128], sg[:, k, cols], ws_r[:, k, :],
                            start=(k == 0), stop=False,
                        )
                    nc.tensor.matmul(
                        pos[:, 0:128], xin[:, 0, cols], ident,
                        start=False, stop=True,
                    )
                    # vector part (comp-major in psum) + residual
                    for i in range(3):
                        dst = pos[:, 128 * (1 + i) : 128 * (2 + i)]
                        nc.tensor.matmul(dst, vg[:, i, cols], wv_r[:, 0, :],
                                         start=True, stop=False)
                        nc.tensor.matmul(dst, svg[:, i, cols], wv_r[:, 1, :],
                                         start=False, stop=False)
                        nc.tensor.matmul(dst, xin[:, 3 + i, cols], ident,
                                         start=False, stop=True)
                    # evacuate raw values to SBUF fp16; alternate the v-part
                    # engine (DVE/ACT) to balance load
                    vdst = y_sb[:, b, 128:512].rearrange("p (o i) -> p i o", i=3)
                    vsrc = pos[:, 128:512].rearrange("p (i o) -> p i o", o=128)
                    nc.scalar.activation(
                        out=y_sb[:, b, 0:128], in_=pos[:, 0:128], func=AF.Copy,
                    )
                    nc.scalar.activation(out=vdst, in_=vsrc, func=AF.Copy)

            def S_stat(t, half):
                d = st[t]
                y_sb = d["y_sb"]
                if half == 0:
                    # w[:, 0:NB] = var_s (filled in S_pmath); w[:, NB:] = msv
                    nw = psmall.tile([128, 2 * NB], F32, tag="nw")
                    st6 = psmall.tile([128, NB, 6], F32, tag="st6")
                    st6v = psmall.tile([128, NB, 6], F32, tag="st6v")
                    d["w"], d["stats6"], d["stats6v"] = nw, st6, st6v
                w, stats6, stat6v = d["w"], d["stats6"], d["stats6v"]
                for b in (2 * half, 2 * half + 1):
                    nc.vector.bn_stats(out=stat6v[:, b, :], in_=y_sb[:, b, 128:512])
                    nc.vector.bn_stats(out=stats6[:, b, :], in_=y_sb[:, b, 0:128])

            def S_pmath(t):
                d = st[t]
                w, stats6, stat6v = d["w"], d["stats6"], d["stats6v"]
                v = nc.vector
                # msv = sumsq_v/128 = (cve+cvo)/128 + 1.5*(me^2+mo^2)
                vme = stat6v[:, :, 1]
                vmo = stat6v[:, :, 4]
                vcve = stat6v[:, :, 2]
                vcvo = stat6v[:, :, 5]
                t1 = psmall.tile([128, NB], F32, tag="t1")
                v.tensor_mul(t1, vme, vme)
                t2 = psmall.tile([128, NB], F32, tag="t2")
                v.tensor_mul(t2, vmo, vmo)
                v.tensor_add(t1, t1, t2)
                v.tensor_scalar(out=t1, in0=t1, scalar1=1.5, scalar2=None,
                                op0=OP.mult)
                t3 = psmall.tile([128, NB], F32, tag="t3")
                v.tensor_add(t3, vcve, vcvo)
                v.scalar_tensor_tensor(out=w[:, NB : 2 * NB], in0=t3,
                                       scalar=1.0 / 128.0, in1=t1,
                                       op0=OP.mult, op1=OP.add)
                me = stats6[:, :, 1]
                mo = stats6[:, :, 4]
                cve = stats6[:, :, 2]
                cvo = stats6[:, :, 5]
                mu2 = psmall.tile([128, NB], F32, tag="mu2")
                v.tensor_add(mu2, me, mo)                 # 2*mu
                dh = psmall.tile([128, NB], F32, tag="dh")
                v.tensor_sub(dh, me, mo)
                v.tensor_scalar(out=dh, in0=dh, scalar1=0.5, scalar2=None,
                                op0=OP.mult)
                d2 = psmall.tile([128, NB], F32, tag="d2")
                v.tensor_mul(d2, dh, dh)
                cv = psmall.tile([128, NB], F32, tag="cv")
                v.tensor_add(cv, cve, cvo)
                # w[:, 0:NB] = var_s = cv/128 + d2 (w[:, NB:] set by ttr)
                v.scalar_tensor_tensor(out=w[:, 0:NB], in0=cv, scalar=1.0 / 128.0,
                                       in1=d2, op0=OP.mult, op1=OP.add)
                # Newton rsqrt seed (int bit trick)
                yv = psmall.tile([128, 2 * NB], F32, tag="ny")
                yi = yv.bitcast(I32)
                wi = w.bitcast(I32)
                v.tensor_scalar(out=yi, in0=wi, scalar1=1, scalar2=None,
                                op0=OP.arith_shift_right)
                v.tensor_scalar(out=yi, in0=yi, scalar1=0x5F3759E0,
                                scalar2=None, op0=OP.subtract)
                v.tensor_scalar(out=yi, in0=yi, scalar1=-1, scalar2=None,
                                op0=OP.bitwise_xor)
                tmp = psmall.tile([128, 2 * NB], F32, tag="nt")
                for _ in range(OPTS["newton_iters"]):
                    v.tensor_mul(tmp, yv, yv)
                    v.tensor_mul(tmp, tmp, w)
                    v.tensor_scalar(out=tmp, in0=tmp, scalar1=-0.5, scalar2=1.5,
                                    op0=OP.mult, op1=OP.add)
                    v.tensor_mul(yv, yv, tmp)
                # beta = -mu*inv_s = -0.5*mu2*inv_s
                beta = psmall.tile([128, NB], F32, tag="beta")
                v.scalar_tensor_tensor(out=beta, in0=mu2, scalar=-0.5,
                                       in1=yv[:, 0:NB], op0=OP.mult, op1=OP.mult)
                d["beta"], d["inv"] = beta, yv

            def S_fin(t):
                d = st[t]
                y_sb, beta, inv = d["y_sb"], d["beta"], d["inv"]
                for b in range(NB):
                    nc.vector.tensor_scalar(
                        out=y_sb[:, b, 0:128], in0=y_sb[:, b, 0:128],
                        scalar1=inv[:, b : b + 1], scalar2=None, op0=OP.mult,
                    )
                    nc.vector.tensor_scalar(
                        out=y_sb[:, b, 0:128], in0=y_sb[:, b, 0:128],
                        scalar1=beta[:, b : b + 1], scalar2=None, op0=OP.add,
                    )
                    nc.vector.tensor_scalar(
                        out=y_sb[:, b, 128:512], in0=y_sb[:, b, 128:512],
                        scalar1=inv[:, NB + b : NB + b + 1], scalar2=None,
                        op0=OP.mult,
                    )

            def S_out(t):
                d = st.pop(t)
                ns = slice(t * T, (t + 1) * T)
                y_blk = y[ns].rearrange("(b p) f -> p b f", p=128)
                nc.sync.dma_start(out=y_blk, in_=d["y_sb"])

            nc.sync.dma_start(out=w1_r, in_=w1[:, :, :])
            S_in(0, split=True)
            S_mm1(0)
            d0 = st[0]
            nc.sync.dma_start(out=d0["xin"][:, 3:6, :], in_=xt_r[:, 3:6, d0.pop("pending")])
            nc.sync.dma_start(out=w2_r, in_=w2[:, :, :])
            S_in(1)
            nc.sync.dma_start(out=ws_r, in_=ws[:, :, :])
            nc.sync.dma_start(out=wv_r, in_=wv[:, :, :])
            nc.sync.dma_start(out=ident, in_=idn[:, :])
            for i in range(NT + 2):
                if i + 2 <= NT - 1:
                    S_in(i + 2)
                if i + 1 <= NT - 1:
                    S_mm1(i + 1)
                if i <= NT - 1:
                    S_mm2a(i)
                if 0 <= i - 1 <= NT - 1:
                    S_lin_half(i - 1, 0)
                if i <= NT - 1:
                    S_mm2b(i)
                if 0 <= i - 1 <= NT - 1:
                    S_lin_half(i - 1, 1)
                    S_stat(i - 1, 0)
                if i <= NT - 1:
                    S_gate(i)
                if 0 <= i - 1 <= NT - 1:
                    S_stat(i - 1, 1)
                    S_pmath(i - 1)
                if 0 <= i - 2 <= NT - 1:
                    S_fin(i - 2)
                    S_out(i - 2)

    nc.finalize()
    return nc


def host_prep(x_full, mlp_w1, mlp_w2, lin_ws, lin_wv):
    x_full = np.asarray(x_full, np.float32)
    n = x_full.shape[0]
    xp = np.zeros((N_CORES * NPC, 512), dtype=np.float32)
    xp[:n] = x_full

    w1 = np.asarray(mlp_w1, np.float32)
    w2 = np.asarray(mlp_w2, np.float32)[:, :640]
    ws_ = np.asarray(lin_ws, np.float32)
    wv_np = np.asarray(lin_wv, np.float32)
    wv_ = np.concatenate(
        [wv_np[:128], np.float32(np.sqrt(2.0)) * wv_np[128:]], axis=0
    )
    # pre-chunk weights along K into [128, k, m] fp16
    w1_r = np.ascontiguousarray(w1.reshape(3, 128, 384).transpose(1, 0, 2)).astype(np.float16)
    w2_r = np.ascontiguousarray(w2.reshape(3, 128, 640).transpose(1, 0, 2)).astype(np.float16)
    ws_r = np.ascontiguousarray(ws_.reshape(3, 128, 128).transpose(1, 0, 2)).astype(np.float16)
    wv_r = np.ascontiguousarray(wv_.reshape(2, 128, 128).transpose(1, 0, 2)).astype(np.float16)
    idn = np.eye(128, dtype=np.float16)

    maps = []
    for c in range(N_CORES):
        xs = xp[c * NPC : (c + 1) * NPC]
        s = xs[:, :128]
        v = xs[:, 128:].reshape(NPC, 128, 3)
        s16 = s.T.astype(np.float16).astype(np.float32)
        v16 = v.astype(np.float16).astype(np.float32)
        xtc = np.empty((6, 128, NPC), dtype=np.float16)
        xtc[0] = s.T
        xtc[1] = (s16 * s16).astype(np.float16)
        xtc[2] = np.sum(v16 * v16, axis=-1).T.astype(np.float16)
        xtc[3] = v[:, :, 0].T
        xtc[4] = v[:, :, 1].T
        xtc[5] = v[:, :, 2].T
        maps.append(dict(xt=xtc, w1=w1_r, w2=w2_r, ws=ws_r, wv=wv_r, idn=idn))
    return maps


_CACHE = {}


def _get_nc():
    if "nc" not in _CACHE:
        _CACHE["nc"] = build_nc()
    return _CACHE["nc"]


def kernel(x, mlp_w1, mlp_w2, lin_ws, lin_wv):
    maps = host_prep(x, mlp_w1, mlp_w2, lin_ws, lin_wv)
    nc = _get_nc()
    res = run_bass_kernel_spmd(nc, maps, list(range(N_CORES)))
    n = np.asarray(x).shape[0]
    out = np.concatenate(
        [res.results[c]["y"] for c in range(N_CORES)], axis=0
    )[:n].astype(np.float32)
    return np.ascontiguousarray(out)


def timed_stats():
    try:
        from concourse.timeline_sim import TimelineSim

        sim = TimelineSim(_get_nc())
        return float(sim.simulate())
    except Exception as e:  # pragma: no cover
        print("timeline sim failed:", e)
        return None


# revision 4
# speedup vs baseline: 1.0291x; 1.0029x over previous
"""TRN2 Bass/Tile kernel v2: nn_ChannelWiseTensorSquareSelfInteraction.

Contract: kernel(**inputs) takes FULL unsharded fp32 inputs
(x [100000,512], mlp_w1 [384,384], mlp_w2 [384,768], lin_ws [384,128],
lin_wv [256,128]) and returns the FULL fp32 output [100000,512].

Strategy (8 cores, data-parallel over nodes, fp16 device pipeline):
  Host ships fp16 feature-major x (s|vx|vy|vz), fp16 products (ss, vv),
  fp16 weights pre-chunked for the PE, fp16 identity; reads back fp16 y.
  Device per 512-node tile:
    mm1 (fp16) -> silu_h (ACT, one call) -> mm2 (fp16, m-split 3+2)
    -> silu_g -> gating muls (DVE + Pool) -> node-major linear with
    residual via identity-rhs matmuls (PE) -> evac (ACT s-part copy,
    DVE v-part interleaving copy) -> stats (DVE: grouped bn_stats for s,
    tensor_tensor_reduce for v sumsq) -> Newton rsqrt + stat algebra
    (Pool, batched) -> per-block (raw-mu)*inv via tensor_scalar with
    per-partition scalar APs (DVE fast mode) -> fp16 store.
  Emission is software-pipelined (mm1 of tile t+1 before mm2 of tile t
  before lin of tile t-1) so the PE — the bottleneck engine at fp16 —
  never waits on ACT/DVE stages.
"""

import numpy as np

import concourse.bacc as bacc
import concourse.mybir as mybir
from concourse.tile import TileContext
from concourse.bass_utils import run_bass_kernel_spmd

F32 = mybir.dt.float32
F16 = mybir.dt.float16
I32 = mybir.dt.int32
AF = mybir.ActivationFunctionType
OP = mybir.AluOpType

N_FULL = 100000
N_CORES = 8
NPC = 12544       # 24 tiles of 512 + 1 tail tile of 256
T = 512
NB = 4            # node blocks per full tile
NT = 25

MAGIC = 0x5F3759DF

OPTS = dict(
    newton_iters=1,
    svg_pool=3,        # how many of the 3 svg muls run on Pool
    evac_s_act=True,   # s-part evacuation on ACT (else DVE)
)


def build_nc():
    nc = bacc.Bacc("TRN2", target_bir_lowering=False, debug=False, num_devices=N_CORES)
    xt = nc.declare_dram_parameter("xt", [6, 128, NPC], F16, isOutput=False)
    w1 = nc.declare_dram_parameter("w1", [128, 3, 384], F16, isOutput=False)
    w2 = nc.declare_dram_parameter("w2", [128, 3, 640], F16, isOutput=False)
    ws = nc.declare_dram_parameter("ws", [128, 3, 128], F16, isOutput=False)
    wv = nc.declare_dram_parameter("wv", [128, 2, 128], F16, isOutput=False)
    idn = nc.declare_dram_parameter("idn", [128, 128], F16, isOutput=False)
    y = nc.declare_dram_parameter("y", [NPC, 512], F16, isOutput=True)

    xt_r = xt.rearrange("c p n -> p c n")

    with TileContext(nc) as tc:
        with (
            tc.tile_pool(name="singles", bufs=1) as singles,
            tc.tile_pool(name="pin", bufs=5) as pin,
            tc.tile_pool(name="pmid", bufs=3) as pmid,
            tc.tile_pool(name="pout", bufs=4) as pout,
            tc.tile_pool(name="psmall", bufs=4) as psmall,
            tc.tile_pool(name="phg", bufs=2, space="PSUM") as phg,
            tc.tile_pool(name="po", bufs=2, space="PSUM") as po,
        ):
            w1_r = singles.tile([128, 3, 384], F16)
            w2_r = singles.tile([128, 3, 640], F16)
            ws_r = singles.tile([128, 3, 128], F16)
            wv_r = singles.tile([128, 2, 128], F16)
            ident = singles.tile([128, 128], F16)
            warm = singles.tile([128, 1], F32)
            nc.vector.memset(warm, 0.0)
            nc.scalar.activation(out=warm, in_=warm, func=AF.Silu)

            # pipeline state, keyed by tile index
            st = {}

            def tsz(t):
                return 256 if t == NT - 1 else T

            def nhalves(t):
                return 1 if t == NT - 1 else 2

            def S_in(t, split=False):
                S_ = tsz(t)
                ns = slice(t * T, t * T + S_)
                xin = pin.tile([128, 6, T], F16, tag="xin")
                if split:
                    nc.sync.dma_start(out=xin[:, 0:3, 0:S_], in_=xt_r[:, 0:3, ns])
                    st[t] = dict(xin=xin, pending=ns)
                else:
                    nc.sync.dma_start(out=xin[:, :, 0:S_], in_=xt_r[:, :, ns])
                    st[t] = dict(xin=xin)

            def S_mm1(t):
                d = st[t]
                S_ = tsz(t)
                xin = d["xin"]
                rhs = [xin[:, k, 0:S_] for k in range(3)]
                ph = phg.tile([128, 3, T], F32, tag="hg")
                for m in range(3):
                    for k in range(3):
                        nc.tensor.matmul(
                            ph[:, m, 0:S_], w1_r[:, k, 128 * m : 128 * (m + 1)],
                            rhs[k], start=(k == 0), stop=(k == 2),
                        )
                h16 = pmid.tile([128, 3, T], F16, tag="h16")
                nc.scalar.activation(
                    out=h16[:, :, 0:S_], in_=ph[:, :, 0:S_], func=AF.Silu,
                )
                d["h16"] = h16

            def S_mm2a(t):
                d = st[t]
                S_ = tsz(t)
                h16 = d["h16"]
                g16 = pmid.tile([128, 5, T], F16, tag="g16")
                d["g16"] = g16
                pga = phg.tile([128, 3, T], F32, tag="hg")
                for m in range(3):
                    for k in range(3):
                        nc.tensor.matmul(
                            pga[:, m, 0:S_], w2_r[:, k, 128 * m : 128 * (m + 1)],
                            h16[:, k, 0:S_], start=(k == 0), stop=(k == 2),
                        )
                nc.scalar.activation(
                    out=g16[:, 0:3, 0:S_], in_=pga[:, :, 0:S_], func=AF.Silu,
                )

            def S_mm2b(t):
                d = st[t]
                S_ = tsz(t)
                h16 = d["h16"]
                g16 = d["g16"]
                pgb = phg.tile([128, 3, T], F32, tag="hg")
                for m in range(2):
                    for k in range(3):
                        nc.tensor.matmul(
                            pgb[:, m, 0:S_], w2_r[:, k, 128 * (3 + m) : 128 * (4 + m)],
                            h16[:, k, 0:S_], start=(k == 0), stop=(k == 2),
                        )
                nc.scalar.activation(
                    out=g16[:, 3:5, 0:S_], in_=pgb[:, 0:2, 0:S_], func=AF.Silu,
                )

            def S_gate(t):
                d = st[t]
                S_ = tsz(t)
                xin, g16 = d["xin"], d["g16"]
                s = xin[:, 0, 0:S_]
                sg = pmid.tile([128, 3, T], F16, tag="sg")
                nc.vector.tensor_mul(sg[:, :, 0:S_], xin[:, 0:3, 0:S_],
                                     g16[:, 0:3, 0:S_])
                gsv = pmid.tile([128, T], F16, tag="gsv")
                nc.vector.tensor_mul(gsv[:, 0:S_], s, g16[:, 4, 0:S_])
                vg = pmid.tile([128, 3, T], F16, tag="vg")
                for i in range(3):
                    nc.vector.tensor_mul(vg[:, i, 0:S_], xin[:, 3 + i, 0:S_],
                                         g16[:, 3, 0:S_])
                svg = pmid.tile([128, 3, T], F16, tag="svg")
                for i in range(3):
                    nc.gpsimd.tensor_mul(svg[:, i, 0:S_], gsv[:, 0:S_],
                                         xin[:, 3 + i, 0:S_])
                d["sg"], d["vg"], d["svg"] = sg, vg, svg

            def S_lin_half(t, half):
                d = st[t]
                xin, sg, vg, svg = d["xin"], d["sg"], d["vg"], d["svg"]
                if half == 0:
                    y_sb = pout.tile([128, NB, 512], F16, tag="y")
                    d["y_sb"] = y_sb
                else:
                    y_sb = d["y_sb"]
                for b in (2 * half, 2 * half + 1):
                    cols = slice(128 * b, 128 * (b + 1))
                    pos = po.tile([128, 512], F32, tag="po")
                    # scalar part + residual (identity rhs)
                    for k in range(3):
                        nc.tensor.matmul(
                            pos[:, 0:128], sg[:, k, cols], ws_r[:, k, :],
                            start=(k == 0), stop=False,
                        )
                    nc.tensor.matmul(
                        pos[:, 0:128], xin[:, 0, cols], ident,
                        start=False, stop=True,
                    )
                    # vector part (comp-major in psum) + residual
                    for i in range(3):
                        dst = pos[:, 128 * (1 + i) : 128 * (2 + i)]
                        nc.tensor.matmul(dst, vg[:, i, cols], wv_r[:, 0, :],
                                         start=True, stop=False)
                        nc.tensor.matmul(dst, svg[:, i, cols], wv_r[:, 1, :],
                                         start=False, stop=False)
                        nc.tensor.matmul(dst, xin[:, 3 + i, cols], ident,
                                         start=False, stop=True)
                    # evacuate raw values to SBUF fp16; alternate the v-part
                    # engine (DVE/ACT) to balance load
                    vdst = y_sb[:, b, 128:512].rearrange("p (o i) -> p i o", i=3)
                    vsrc = pos[:, 128:512].rearrange("p (i o) -> p i o", o=128)
                    nc.scalar.activation(
                        out=y_sb[:, b, 0:128], in_=pos[:, 0:128], func=AF.Copy,
                    )
                    nc.scalar.activation(out=vdst, in_=vsrc, func=AF.Copy)

            def S_stat(t, half):
                d = st[t]
                y_sb = d["y_sb"]
                if half == 0:
                    # w[:, 0:nbt] = var_s (filled in S_pmath); w[:, nbt:2nbt] = msv
                    nw = psmall.tile([128, 2 * NB], F32, tag="nw")
                    st6 = psmall.tile([128, NB, 6], F32, tag="st6")
                    st6v = psmall.tile([128, NB, 6], F32, tag="st6v")
                    d["w"], d["stats6"], d["stats6v"] = nw, st6, st6v
                w, stats6, stat6v = d["w"], d["stats6"], d["stats6v"]
                for b in (2 * half, 2 * half + 1):
                    nc.vector.bn_stats(out=stat6v[:, b, :], in_=y_sb[:, b, 128:512])
                    nc.vector.bn_stats(out=stats6[:, b, :], in_=y_sb[:, b, 0:128])

            def S_pmath(t):
                d = st[t]
                nbt = 2 * nhalves(t)
                w, stats6, stat6v = d["w"], d["stats6"], d["stats6v"]
                v = nc.vector
                # msv = sumsq_v/128 = (cve+cvo)/128 + 1.5*(me^2+mo^2)
                vme = stat6v[:, 0:nbt, 1]
                vmo = stat6v[:, 0:nbt, 4]
                vcve = stat6v[:, 0:nbt, 2]
                vcvo = stat6v[:, 0:nbt, 5]
                t1 = psmall.tile([128, NB], F32, tag="t1")
                v.tensor_mul(t1[:, 0:nbt], vme, vme)
                t2 = psmall.tile([128, NB], F32, tag="t2")
                v.tensor_mul(t2[:, 0:nbt], vmo, vmo)
                v.tensor_add(t1[:, 0:nbt], t1[:, 0:nbt], t2[:, 0:nbt])
                v.tensor_scalar(out=t1[:, 0:nbt], in0=t1[:, 0:nbt], scalar1=1.5,
                                scalar2=None, op0=OP.mult)
                t3 = psmall.tile([128, NB], F32, tag="t3")
                v.tensor_add(t3[:, 0:nbt], vcve, vcvo)
                v.scalar_tensor_tensor(out=w[:, nbt : 2 * nbt], in0=t3[:, 0:nbt],
                                       scalar=1.0 / 128.0, in1=t1[:, 0:nbt],
                                       op0=OP.mult, op1=OP.add)
                me = stats6[:, 0:nbt, 1]
                mo = stats6[:, 0:nbt, 4]
                cve = stats6[:, 0:nbt, 2]
                cvo = stats6[:, 0:nbt, 5]
                mu2 = psmall.tile([128, NB], F32, tag="mu2")
                v.tensor_add(mu2[:, 0:nbt], me, mo)       # 2*mu
                dh = psmall.tile([128, NB], F32, tag="dh")
                v.tensor_sub(dh[:, 0:nbt], me, mo)
                v.tensor_scalar(out=dh[:, 0:nbt], in0=dh[:, 0:nbt], scalar1=0.5,
                                scalar2=None, op0=OP.mult)
                d2 = psmall.tile([128, NB], F32, tag="d2")
                v.tensor_mul(d2[:, 0:nbt], dh[:, 0:nbt], dh[:, 0:nbt])
                cv = psmall.tile([128, NB], F32, tag="cv")
                v.tensor_add(cv[:, 0:nbt], cve, cvo)
                # w[:, 0:nbt] = var_s = cv/128 + d2
                v.scalar_tensor_tensor(out=w[:, 0:nbt], in0=cv[:, 0:nbt],
                                       scalar=1.0 / 128.0, in1=d2[:, 0:nbt],
                                       op0=OP.mult, op1=OP.add)
                # Newton rsqrt seed (int bit trick)
                nw2 = 2 * nbt
                yv = psmall.tile([128, 2 * NB], F32, tag="ny")
                yi = yv.bitcast(I32)
                wi = w.bitcast(I32)
                v.tensor_scalar(out=yi[:, 0:nw2], in0=wi[:, 0:nw2], scalar1=1,
                                scalar2=None, op0=OP.arith_shift_right)
                v.tensor_scalar(out=yi[:, 0:nw2], in0=yi[:, 0:nw2],
                                scalar1=0x5F3759E0, scalar2=None, op0=OP.subtract)
                v.tensor_scalar(out=yi[:, 0:nw2], in0=yi[:, 0:nw2], scalar1=-1,
                                scalar2=None, op0=OP.bitwise_xor)
                tmp = psmall.tile([128, 2 * NB], F32, tag="nt")
                for _ in range(OPTS["newton_iters"]):
                    v.tensor_mul(tmp[:, 0:nw2], yv[:, 0:nw2], yv[:, 0:nw2])
                    v.tensor_mul(tmp[:, 0:nw2], tmp[:, 0:nw2], w[:, 0:nw2])
                    v.tensor_scalar(out=tmp[:, 0:nw2], in0=tmp[:, 0:nw2],
                                    scalar1=-0.5, scalar2=1.5,
                                    op0=OP.mult, op1=OP.add)
                    v.tensor_mul(yv[:, 0:nw2], yv[:, 0:nw2], tmp[:, 0:nw2])
                # beta = -mu*inv_s = -0.5*mu2*inv_s
                beta = psmall.tile([128, NB], F32, tag="beta")
                v.scalar_tensor_tensor(out=beta[:, 0:nbt], in0=mu2[:, 0:nbt],
                                       scalar=-0.5, in1=yv[:, 0:nbt],
                                       op0=OP.mult, op1=OP.mult)
                d["beta"], d["inv"] = beta, yv

            def S_fin(t):
                d = st[t]
                nbt = 2 * nhalves(t)
                y_sb, beta, inv = d["y_sb"], d["beta"], d["inv"]
                for b in range(nbt):
                    nc.vector.tensor_scalar(
                        out=y_sb[:, b, 0:128], in0=y_sb[:, b, 0:128],
                        scalar1=inv[:, b : b + 1], scalar2=None, op0=OP.mult,
                    )
                    nc.vector.tensor_scalar(
                        out=y_sb[:, b, 0:128], in0=y_sb[:, b, 0:128],
                        scalar1=beta[:, b : b + 1], scalar2=None, op0=OP.add,
                    )
                    nc.vector.tensor_scalar(
                        out=y_sb[:, b, 128:512], in0=y_sb[:, b, 128:512],
                        scalar1=inv[:, nbt + b : nbt + b + 1], scalar2=None,
                        op0=OP.mult,
                    )

            def S_out(t):
                d = st.pop(t)
                nbt = 2 * nhalves(t)
                ns = slice(t * T, t * T + tsz(t))
                y_blk = y[ns].rearrange("(b p) f -> p b f", p=128)
                nc.sync.dma_start(out=y_blk, in_=d["y_sb"][:, 0:nbt, :])

            nc.sync.dma_start(out=w1_r, in_=w1[:, :, :])
            S_in(0, split=True)
            S_mm1(0)
            d0 = st[0]
            nc.sync.dma_start(out=d0["xin"][:, 3:6, :], in_=xt_r[:, 3:6, d0.pop("pending")])
            nc.sync.dma_start(out=w2_r, in_=w2[:, :, :])
            S_in(1)
            nc.sync.dma_start(out=ws_r, in_=ws[:, :, :])
            nc.sync.dma_start(out=wv_r, in_=wv[:, :, :])
            nc.sync.dma_start(out=ident, in_=idn[:, :])
            for i in range(NT + 2):
                if i + 2 <= NT - 1:
                    S_in(i + 2)
                if i + 1 <= NT - 1:
                    S_mm1(i + 1)
                if i <= NT - 1:
                    S_mm2a(i)
                if 0 <= i - 1 <= NT - 1:
                    S_lin_half(i - 1, 0)
                if i <= NT - 1:
                    S_mm2b(i)
                if 0 <= i - 1 <= NT - 1:
                    if nhalves(i - 1) == 2:
                        S_lin_half(i - 1, 1)
                    S_stat(i - 1, 0)
                if i <= NT - 1:
                    S_gate(i)
                if 0 <= i - 1 <= NT - 1:
                    if nhalves(i - 1) == 2:
                        S_stat(i - 1, 1)
                    S_pmath(i - 1)
                if 0 <= i - 2 <= NT - 1:
                    S_fin(i - 2)
                    S_out(i - 2)

    nc.finalize()
    return nc


def host_prep(x_full, mlp_w1, mlp_w2, lin_ws, lin_wv):
    x_full = np.asarray(x_full, np.float32)
    n = x_full.shape[0]
    xp = np.zeros((N_CORES * NPC, 512), dtype=np.float32)
    xp[:n] = x_full

    w1 = np.asarray(mlp_w1, np.float32)
    w2 = np.asarray(mlp_w2, np.float32)[:, :640]
    ws_ = np.asarray(lin_ws, np.float32)
    wv_np = np.asarray(lin_wv, np.float32)
    wv_ = np.concatenate(
        [wv_np[:128], np.float32(np.sqrt(2.0)) * wv_np[128:]], axis=0
    )
    # pre-chunk weights along K into [128, k, m] fp16
    w1_r = np.ascontiguousarray(w1.reshape(3, 128, 384).transpose(1, 0, 2)).astype(np.float16)
    w2_r = np.ascontiguousarray(w2.reshape(3, 128, 640).transpose(1, 0, 2)).astype(np.float16)
    ws_r = np.ascontiguousarray(ws_.reshape(3, 128, 128).transpose(1, 0, 2)).astype(np.float16)
    wv_r = np.ascontiguousarray(wv_.reshape(2, 128, 128).transpose(1, 0, 2)).astype(np.float16)
    idn = np.eye(128, dtype=np.float16)

    maps = []
    for c in range(N_CORES):
        xs = xp[c * NPC : (c + 1) * NPC]
        s = xs[:, :128]
        v = xs[:, 128:].reshape(NPC, 128, 3)
        s16 = s.T.astype(np.float16).astype(np.float32)
        v16 = v.astype(np.float16).astype(np.float32)
        xtc = np.empty((6, 128, NPC), dtype=np.float16)
        xtc[0] = s.T
        xtc[1] = (s16 * s16).astype(np.float16)
        xtc[2] = np.sum(v16 * v16, axis=-1).T.astype(np.float16)
        xtc[3] = v[:, :, 0].T
        xtc[4] = v[:, :, 1].T
        xtc[5] = v[:, :, 2].T
        maps.append(dict(xt=xtc, w1=w1_r, w2=w2_r, ws=ws_r, wv=wv_r, idn=idn))
    return maps


_CACHE = {}


def _get_nc():
    if "nc" not in _CACHE:
        _CACHE["nc"] = build_nc()
    return _CACHE["nc"]


def kernel(x, mlp_w1, mlp_w2, lin_ws, lin_wv):
    maps = host_prep(x, mlp_w1, mlp_w2, lin_ws, lin_wv)
    nc = _get_nc()
    res = run_bass_kernel_spmd(nc, maps, list(range(N_CORES)))
    n = np.asarray(x).shape[0]
    out = np.concatenate(
        [res.results[c]["y"] for c in range(N_CORES)], axis=0
    )[:n].astype(np.float32)
    return np.ascontiguousarray(out)


def timed_stats():
    try:
        from concourse.timeline_sim import TimelineSim

        sim = TimelineSim(_get_nc())
        return float(sim.simulate())
    except Exception as e:  # pragma: no cover
        print("timeline sim failed:", e)
        return None
